# revision 1
# baseline (speedup 1.0000x reference)
import numpy as np
import jax
import jax.numpy as jnp

HEADS = 8
DIM_HEAD = 64
C = 512
WIN = 7
N = WIN * WIN
EPS = 1e-5
NCORES = 8


def _rel_bias(rel_table: np.ndarray) -> np.ndarray:
    # rel_table [13,13,8] -> bias [8,49,49] (Swin-style)
    hh = np.arange(WIN)
    hi = np.repeat(hh, WIN)
    wi = np.tile(hh, WIN)
    dh = hi[:, None] - hi[None, :] + WIN - 1
    dw = wi[:, None] - wi[None, :] + WIN - 1
    bias = rel_table[dh, dw]  # [49,49,heads]
    return np.ascontiguousarray(np.transpose(bias, (2, 0, 1)))


def _forward(x, gamma, beta, w_qkv, bias, w_out, b_out):
    b = x.shape[0]
    xs = jnp.transpose(x.reshape(b, C, N), (0, 2, 1))  # [b,N,C]
    mu = jnp.mean(xs, axis=-1, keepdims=True)
    var = jnp.var(xs, axis=-1, keepdims=True)
    xn = (xs - mu) * jax.lax.rsqrt(var + EPS) * gamma + beta
    qkv = xn @ w_qkv  # [b,N,3*inner]
    q, k, v = jnp.split(qkv, 3, axis=-1)

    def heads(t):
        return jnp.transpose(t.reshape(b, N, HEADS, DIM_HEAD), (0, 2, 1, 3))

    q, k, v = heads(q), heads(k), heads(v)
    dots = jnp.einsum('bhnd,bhmd->bhnm', q, k) * (DIM_HEAD ** -0.5) + bias[None]
    attn = jax.nn.softmax(dots, axis=-1)
    out = jnp.einsum('bhnm,bhmd->bhnd', attn, v)
    out = jnp.transpose(out, (0, 2, 1, 3)).reshape(b, N, HEADS * DIM_HEAD)
    out = out @ w_out + b_out
    out = jnp.transpose(out, (0, 2, 1)).reshape(b, C, WIN, WIN)
    return out + x


_pforward = None


def _get_pforward():
    global _pforward
    if _pforward is None:
        _pforward = jax.pmap(
            _forward, in_axes=(0, None, None, None, None, None, None)
        )
    return _pforward


def kernel(x, gamma, beta, w_qkv, rel_table, w_out, b_out):
    x = np.asarray(x, dtype=np.float32)
    B = x.shape[0]
    bias = _rel_bias(np.asarray(rel_table, dtype=np.float32))
    ndev = len(jax.devices())
    ncores = NCORES if (NCORES <= ndev and B % NCORES == 0) else 1
    if ncores > 1:
        xs = x.reshape(ncores, B // ncores, C, WIN, WIN)
        out = _get_pforward()(
            xs,
            jnp.asarray(gamma), jnp.asarray(beta), jnp.asarray(w_qkv),
            jnp.asarray(bias), jnp.asarray(w_out), jnp.asarray(b_out),
        )
        out = np.asarray(out).reshape(B, C, WIN, WIN)
    else:
        out = np.asarray(
            jax.jit(_forward)(
                jnp.asarray(x), jnp.asarray(gamma), jnp.asarray(beta),
                jnp.asarray(w_qkv), jnp.asarray(bias), jnp.asarray(w_out),
                jnp.asarray(b_out),
            )
        )
    return out.astype(np.float32)



# revision 2
# speedup vs baseline: 1.8347x; 1.8347x over previous
import numpy as np
import jax
import jax.numpy as jnp
import ml_dtypes

HEADS = 8
DIM_HEAD = 64
C = 512
WIN = 7
N = WIN * WIN
EPS = 1e-5
NCORES = 8
BF16 = ml_dtypes.bfloat16


def _rel_bias(rel_table: np.ndarray) -> np.ndarray:
    # rel_table [13,13,8] -> bias [8,49,49] (Swin-style)
    hh = np.arange(WIN)
    hi = np.repeat(hh, WIN)
    wi = np.tile(hh, WIN)
    dh = hi[:, None] - hi[None, :] + WIN - 1
    dw = wi[:, None] - wi[None, :] + WIN - 1
    bias = rel_table[dh, dw]  # [49,49,heads]
    return np.ascontiguousarray(np.transpose(bias, (2, 0, 1)))


def _forward(xb, gamma, beta, w_qkv, bias, w_out, b_out):
    # xb: [b, C, 7, 7] bf16 on device. Returns attention block output
    # WITHOUT the residual (added on host in fp32), as bf16.
    b = xb.shape[0]
    xs = jnp.transpose(xb.reshape(b, C, N), (0, 2, 1)).astype(jnp.float32)
    mu = jnp.mean(xs, axis=-1, keepdims=True)
    var = jnp.var(xs, axis=-1, keepdims=True)
    xn = (xs - mu) * jax.lax.rsqrt(var + EPS) * gamma + beta
    xn16 = xn.astype(jnp.bfloat16)
    qkv = jnp.matmul(xn16, w_qkv, preferred_element_type=jnp.float32)
    q, k, v = jnp.split(qkv, 3, axis=-1)

    def heads(t):
        return jnp.transpose(
            t.reshape(b, N, HEADS, DIM_HEAD), (0, 2, 1, 3)
        ).astype(jnp.bfloat16)

    q, k, v = heads(q), heads(k), heads(v)
    dots = (
        jnp.einsum('bhnd,bhmd->bhnm', q, k, preferred_element_type=jnp.float32)
        * (DIM_HEAD ** -0.5)
        + bias[None]
    )
    attn = jax.nn.softmax(dots, axis=-1).astype(jnp.bfloat16)
    out = jnp.einsum('bhnm,bhmd->bhnd', attn, v, preferred_element_type=jnp.float32)
    out = jnp.transpose(out, (0, 2, 1, 3)).reshape(b, N, HEADS * DIM_HEAD)
    out = jnp.matmul(
        out.astype(jnp.bfloat16), w_out, preferred_element_type=jnp.float32
    ) + b_out
    out = jnp.transpose(out, (0, 2, 1)).reshape(b, C, WIN, WIN)
    return out.astype(jnp.bfloat16)


_pforward = None


def _get_pforward():
    global _pforward
    if _pforward is None:
        _pforward = jax.pmap(
            _forward, in_axes=(0, None, None, None, None, None, None)
        )
    return _pforward


def kernel(x, gamma, beta, w_qkv, rel_table, w_out, b_out):
    x = np.ascontiguousarray(np.asarray(x, dtype=np.float32))
    B = x.shape[0]
    bias = _rel_bias(np.asarray(rel_table, dtype=np.float32))
    # Halve tunnel traffic: ship activations as bf16, keep weights fp32-
    # derived bf16 (tiny). Residual is added on host in fp32.
    x16 = x.astype(BF16)
    w_qkv16 = np.asarray(w_qkv, dtype=np.float32).astype(BF16)
    w_out16 = np.asarray(w_out, dtype=np.float32).astype(BF16)
    ndev = len(jax.devices())
    ncores = NCORES if (NCORES <= ndev and B % NCORES == 0) else 1
    if ncores > 1:
        xs = x16.reshape(ncores, B // ncores, C, WIN, WIN)
        out = _get_pforward()(
            xs,
            jnp.asarray(gamma, dtype=jnp.float32),
            jnp.asarray(beta, dtype=jnp.float32),
            jnp.asarray(w_qkv16),
            jnp.asarray(bias),
            jnp.asarray(w_out16),
            jnp.asarray(b_out, dtype=jnp.float32),
        )
        out = np.asarray(out, dtype=np.float32).reshape(B, C, WIN, WIN)
    else:
        out = np.asarray(
            jax.jit(_forward)(
                jnp.asarray(x16), jnp.asarray(gamma), jnp.asarray(beta),
                jnp.asarray(w_qkv16), jnp.asarray(bias),
                jnp.asarray(w_out16), jnp.asarray(b_out),
            ),
            dtype=np.float32,
        )
    return out + x
